# revision 9
# baseline (speedup 1.0000x reference)
"""MoD router kernel for Trainium2 (8 NeuronCores, SPMD).

Computation (matches the jax reference):
  logits  = x @ w                      [B, L]
  noisy   = logits + 0.1 * noise
  mask    = per-row top-(L/2) of noisy -> bool [B, L]
  aux     = 0.01 * mean_b((mean_l sigmoid(logits) - 0.5)^2)

Sharding: x flattened to [B*L, D] tokens; core i owns tokens
[i*TOK, (i+1)*TOK).  Each core computes its logit shard (fused
multiply+reduce on the vector engine, overlapped with the 33.5MB HBM
read), the shards are AllGathered (16KB each), then every core
redundantly computes the per-row top-k threshold by counting bisection
(cross-partition count via one matmul against a block-diagonal ones
matrix, which reduces and broadcasts in a single PE instruction), the
full mask, and the aux loss (tanh(x/2) form: sigmoid(x)-0.5 =
0.5*tanh(x/2), which avoids the cancellation in mean(sigmoid)-0.5).

Outputs per core: its logits shard [TOK], the full mask [B*L]
(identical on every core; host takes core 0's), and aux [1,1].
"""

import numpy as np

import concourse.bacc as bacc
import concourse.tile as tile
import concourse.mybir as mybir
from concourse import library_config
from concourse.bass_utils import run_bass_kernel_spmd

F32 = mybir.dt.float32
ALU = mybir.AluOpType
ACTF = mybir.ActivationFunctionType

# Full-size problem config (hardcoded per the harness contract).
B, L, D = 4, 8192, 2048
N_CORES = 8
CAP = L // 2                     # top-k capacity per row
TCOL = 4                         # logit columns per x DMA tile (4MB tiles)
NITER = 32                       # bisection iterations
W0 = 0.25                        # initial bracket = [-W0, +W0]


def build_nc(b, l, d, n_cores, cap, tcol, niter, w0, stages="AGBC"):
    """Build the SPMD Bass program. Generic in the shape parameters so a
    scaled-down instance can run under the multi-core simulator.
    `stages`: subset of "AGBC" (matvec / AllGather / bisect+mask / aux) plus
    optional "n" = skip the bisection loop inside B. For debugging."""
    tok = b * l // n_cores        # tokens per core
    cols = tok // 128             # logit columns per core (token = p*cols + c)
    fb = b * l // 128             # free size of the full [128, fb] token layout
    pb = 128 // b                 # partitions per row in the full layout
    assert tok % 128 == 0 and cols % tcol == 0 and (b * l) % 128 == 0
    assert l % fb == 0 or True

    nc = bacc.Bacc("TRN2", target_bir_lowering=False, num_devices=n_cores)

    x = nc.dram_tensor("x_shard", [tok, d], F32, kind="ExternalInput")
    noise = nc.dram_tensor("noise", [b, l], F32, kind="ExternalInput")
    w = nc.dram_tensor("w", [d], F32, kind="ExternalInput")
    logits_out = nc.dram_tensor("logits_out", [tok], F32, kind="ExternalOutput")
    mask_out = nc.dram_tensor("mask_out", [b * l], F32, kind="ExternalOutput")
    aux_out = nc.dram_tensor("aux_out", [1, 1], F32, kind="ExternalOutput")

    # token t of this core = p*cols + c  ->  x row
    xv = x.ap().rearrange("(p c) d -> p c d", p=128)
    lv = logits_out.ap().rearrange("(p c) -> p c", p=128)
    mv = mask_out.ap().rearrange("(p f) -> p f", p=128)
    nv = noise.ap().rearrange("b (q f) -> (b q) f", f=fb)   # [128, fb] global order
    wv = w.ap().rearrange("(o d) -> o d", o=1)

    with tile.TileContext(nc, num_cores=n_cores) as tc:
        with (
            tc.tile_pool(name="consts", bufs=1) as cpool,
            tc.tile_pool(name="xp", bufs=3) as xpool,
            tc.tile_pool(name="scr", bufs=2) as spool,
            tc.tile_pool(name="keep", bufs=1) as kpool,
            tc.tile_pool(name="iter", bufs=2) as ipool,
            tc.tile_pool(name="psum", bufs=2, space="PSUM") as pspool,
            tc.tile_pool(name="dram", bufs=1, space="DRAM") as dpool,
        ):
            # ---- constants ----
            nc.gpsimd.load_library(library_config.mlp)
            w_row = cpool.tile([1, d], F32)
            nc.sync.dma_start(w_row[:, :], wv)
            w_b = cpool.tile([128, d], F32)
            nc.gpsimd.partition_broadcast(w_b[:, :], w_row[:, :])

            bd = cpool.tile([128, 128], F32)     # block-diagonal ones
            nc.vector.memset(bd[:, :], 0.0)
            for r in range(b):
                nc.vector.memset(bd[r * pb:(r + 1) * pb, r * pb:(r + 1) * pb], 1.0)
            rowsel = cpool.tile([128, b], F32)   # rowsel[p, r] = 1 iff p//pb == r
            nc.vector.memset(rowsel[:, :], 0.0)
            for r in range(b):
                nc.vector.memset(rowsel[r * pb:(r + 1) * pb, r:r + 1], 1.0)
            ones_b = cpool.tile([b, 1], F32)
            nc.vector.memset(ones_b[:, :], 1.0)

            # ---- phase A: logits shard = x @ w ----
            logits_sb = kpool.tile([128, cols], F32)
            for j0 in range(0, cols, tcol):
                xt = xpool.tile([128, tcol, d], F32, tag="xt")
                nc.sync.dma_start(xt[:, :, :], xv[:, j0:j0 + tcol, :])
                for k in range(tcol):
                    scr = spool.tile([128, d], F32, tag="prod")
                    nc.vector.tensor_mul(scr[:, :], xt[:, k, :], w_b[:, :])
                    nc.scalar.activation(
                        out=scr[:, :], in_=scr[:, :], func=ACTF.Copy,
                        accum_out=logits_sb[:, j0 + k:j0 + k + 1],
                    )
            nc.sync.dma_start(lv, logits_sb[:, :])

            if "G" in stages:
                ag_in = dpool.tile([1, tok], F32)
                ag_out = dpool.tile([n_cores, tok], F32, addr_space="Shared")
                nc.sync.dma_start(
                    ag_in.rearrange("o (p c) -> (o p) c", p=128), logits_sb[:, :]
                )
                nc.gpsimd.collective_compute(
                    "AllGather",
                    ALU.bypass,
                    replica_groups=[list(range(n_cores))],
                    ins=[ag_in[:, :]],
                    outs=[ag_out[:, :]],
                )
                gsb = kpool.tile([128, fb], F32)   # all logits, global order
                nc.sync.dma_start(
                    gsb[:, :],
                    ag_out.rearrange("r t -> (r t)").rearrange("(p f) -> p f", p=128),
                )

            if "B" in stages:
                # ---- phase B: noisy logits + bisection threshold + mask ----
                nsb = kpool.tile([128, fb], F32)
                nc.sync.dma_start(nsb[:, :], nv)
                noisy = kpool.tile([128, fb], F32)
                nc.vector.scalar_tensor_tensor(
                    out=noisy[:, :], in0=nsb[:, :], scalar=0.1, in1=gsb[:, :],
                    op0=ALU.mult, op1=ALU.add,
                )

                lo = kpool.tile([128, 1], F32)
                nc.vector.memset(lo[:, :], -w0)
                thr = float(cap) - 0.5
                if "n" not in stages:
                    for i in range(niter):
                        a_i = float(w0 * (0.5 ** i))
                        m = ipool.tile([128, 1], F32, tag="m")
                        nc.vector.tensor_scalar_add(m[:, :], lo[:, :], a_i)
                        cmp = ipool.tile([128, fb], F32, tag="cmp")
                        cnt = ipool.tile([128, 1], F32, tag="cnt")
                        nc.vector.tensor_scalar(
                            cmp[:, :], noisy[:, :], m[:, :], None, ALU.is_gt,
                            ALU.add, accum_out=cnt[:, :],
                        )
                        tot = pspool.tile([128, 1], F32, tag="tot")
                        nc.tensor.matmul(tot[:, :], lhsT=bd[:, :], rhs=cnt[:, :],
                                         start=True, stop=True)
                        step = ipool.tile([128, 1], F32, tag="step")
                        nc.vector.tensor_scalar(
                            step[:, :], tot[:, :], thr, a_i, ALU.is_ge, ALU.mult,
                        )
                        nc.vector.tensor_add(lo[:, :], lo[:, :], step[:, :])

                msk = kpool.tile([128, fb], F32)
                nc.vector.tensor_scalar(
                    msk[:, :], noisy[:, :], lo[:, :], None, ALU.is_gt,
                )
                nc.sync.dma_start(mv, msk[:, :])

            if "C" in stages:
                # ---- phase C: aux loss ----
                th = spool.tile([128, fb], F32, tag="prod")
                tsum = kpool.tile([128, 1], F32)
                nc.scalar.activation(
                    out=th[:, :], in_=gsb[:, :], func=ACTF.Tanh, scale=0.5,
                    accum_out=tsum[:, :],
                )
                rs = pspool.tile([b, 1], F32, tag="rs")
                nc.tensor.matmul(rs[:, :], lhsT=rowsel[:, :], rhs=tsum[:, :],
                                 start=True, stop=True)
                t4 = kpool.tile([b, 1], F32)
                nc.vector.tensor_scalar(t4[:, :], rs[:, :], float(0.5 / l), None, ALU.mult)
                t4sq = kpool.tile([b, 1], F32)
                nc.vector.tensor_mul(t4sq[:, :], t4[:, :], t4[:, :])
                auxp = pspool.tile([1, 1], F32, tag="auxp")
                nc.tensor.matmul(auxp[:, :], lhsT=ones_b[:, :], rhs=t4sq[:, :],
                                 start=True, stop=True)
                aux_sb = kpool.tile([1, 1], F32)
                nc.vector.tensor_scalar(aux_sb[:, :], auxp[:, :], float(0.01 / b), None, ALU.mult)
                nc.sync.dma_start(aux_out.ap(), aux_sb[:, :])

    nc.compile()
    return nc


_NC_CACHE = {}


def _get_nc():
    key = (B, L, D, N_CORES, CAP, TCOL, NITER, W0)
    if key not in _NC_CACHE:
        _NC_CACHE[key] = build_nc(B, L, D, N_CORES, CAP, TCOL, NITER, W0)
    return _NC_CACHE[key]


def run(inputs, trace=False):
    x = np.ascontiguousarray(inputs["x"], dtype=np.float32)
    noise = np.ascontiguousarray(inputs["noise"], dtype=np.float32)
    w = np.ascontiguousarray(inputs["w"], dtype=np.float32)
    tok = B * L // N_CORES
    x_flat = x.reshape(B * L, D)
    in_maps = [
        {
            "x_shard": np.ascontiguousarray(x_flat[r * tok:(r + 1) * tok]),
            "noise": noise,
            "w": w,
        }
        for r in range(N_CORES)
    ]
    nc = _get_nc()
    res = run_bass_kernel_spmd(nc, in_maps, core_ids=list(range(N_CORES)),
                               trace=trace)
    logits = np.concatenate(
        [res.results[r]["logits_out"] for r in range(N_CORES)]
    ).reshape(B, L)
    maskf = res.results[0]["mask_out"].reshape(B, L)
    mask = maskf > 0.5
    aux = np.float32(res.results[0]["aux_out"][0, 0])
    return (mask, logits, aux), res


def kernel(**inputs):
    (mask, logits, aux), _ = run(inputs, trace=False)
    return mask, logits, aux
